# revision 31
# baseline (speedup 1.0000x reference)
"""FP8Linear kernel for Trainium2 (Bass/Tile), distributed over 8 NeuronCores.

Computation (matches the nn.Module reference):
    x:  [B=2, S=4096, K=4096] f32  -> x2d [M=8192, K]
    xq = tile_quant_dequant(x2d)    # per-row 1x64 chunks, fp8 e4m3fn round-trip
    wq = block_quant_dequant(w)     # 64x64 blocks of w [N=4096, K]
    out = f32(bf16(xq @ wq.T)) + bias  -> [B, S, N]

Distribution: 8x1 grid -- pure data-parallel over M. Each core takes 1024 x
rows + the full weight and computes a [1024, 4096] output shard. No
collectives, no DRAM operand scratch.

Per-core dataflow (all-SBUF):
  - x quantized once into 8 resident xT units [128k, 33, 128m] bf16 via
    per-row 1x64 amax -> s2 = max(amax,1e-12)/224 -> q = fp8e4(x*rs2) ->
    dq = bf16(q*s2), then ONE SBUF->SBUF xbar DMA transpose per unit.
  - w quantized panel-by-panel (512 out_features ahead of the matmul
    sweeps) with 64x64 block scales. The scale stage (amax, PE-transpose,
    group reduce, s2 and 1/s2 on the tiny [64,2] tile, DRAM-bounce
    partition broadcast) runs ONE ROW TILE AHEAD of the q/dq stage so the
    bounce latency hides under a sweep.
  - The 33rd k-slice carries the bias: xT slice = e0 (ones on partition 0),
    wT slice = bias row on partition 0 -> the PSUM accumulation adds b[n].
  - Sweeps: for panel, for m-tile: 33 bf16 matmuls accumulate [128,512]
    f32 in PSUM; ACT evacuates with a bf16 cast straight to the DRAM
    output (host casts to f32).
Engine split: DVE amax/scales/q-mult, GpSimd dq-mult, ACT evac + all
xbar transposes, Sync ring loads/stores/scale-bounce.
"""

import sys

sys.path.insert(0, "/opt/trn_rl_repo")

import numpy as np
from contextlib import ExitStack

import concourse.bass as bass
import concourse.mybir as mybir
import concourse.tile as tile
from concourse import bacc
from concourse.bass import ts
from concourse.masks import make_identity

P = 128
QT = 64  # quantization tile (1x64 for x, 64x64 for w)

# full-problem dims
B, S, K, N = 2, 4096, 4096, 4096
M = B * S
N_CORES = 8
M_SH = M // N_CORES  # 1024 rows of x per core

KB = K // P          # 32 k-slices of 128
KB1 = KB + 1         # +1 bias slice
KBF = K // QT        # 64 scale columns
NH = 2               # halves per 4096-wide row tile
KH = K // NH         # 2048
KBH = KBF // NH      # 32 scale cols per half

N_PANEL = 512
PANELS = N // N_PANEL    # 8
MTILES = M_SH // P       # 8
WR = N_PANEL // P        # 4 w row tiles per panel

F32 = mybir.dt.float32
BF16 = mybir.dt.bfloat16
FP8 = mybir.dt.float8e4


class Ctx:
    """Bag of pools / constants shared by the emit helpers."""


def _load_amax(cx, nc, src, row0):
    """Load one [128, 4096] f32 row tile of `src` in halves and compute the
    1x64 chunk amax. Returns (nat half tiles, amax [128, KBF] tile)."""
    nats = []
    a = cx.amax.tile([P, KBF], F32, tag="amax")
    for hh in range(NH):
        nat = cx.nat.tile([P, KH], F32, tag="nat")
        nats.append(nat)
        nc.sync.dma_start(nat[:], src[row0 : row0 + P, hh * KH : (hh + 1) * KH])
        nc.vector.tensor_reduce(
            a[:, hh * KBH : (hh + 1) * KBH],
            nat[:].rearrange("p (c t) -> p c t", t=QT),
            axis=mybir.AxisListType.X, op=mybir.AluOpType.max,
            apply_absolute_value=True,
        )
    return nats, a


def _qdq(cx, nc, nats, dq, s2ap, rs2ap):
    """fp8 round-trip of the loaded halves into dq [128, 4096] bf16.
    s2ap/rs2ap: [P, KBF] access patterns (possibly strided)."""
    for hh in range(NH):
        q = cx.q.tile([P, KH], FP8, tag="q")
        q_v = q[:].rearrange("p (c t) -> p c t", t=QT)
        nc.vector.tensor_tensor(
            q_v, nats[hh][:].rearrange("p (c t) -> p c t", t=QT),
            rs2ap[:, hh * KBH : (hh + 1) * KBH, None].to_broadcast((P, KBH, QT)),
            op=mybir.AluOpType.mult,
        )
        nc.gpsimd.tensor_tensor(
            dq[:, hh * KH : (hh + 1) * KH].rearrange("p (c t) -> p c t", t=QT),
            q_v,
            s2ap[:, hh * KBH : (hh + 1) * KBH, None].to_broadcast((P, KBH, QT)),
            op=mybir.AluOpType.mult,
        )


def _emit_x_unit(cx, nc, x, mi):
    """Quantize x rows [mi*128, +128) and transpose into resident xT unit."""
    xT = cx.xT.tile([P, KB, P], BF16, tag="xT", bufs=MTILES, name=f"xT{mi}")
    cx.xT_units[mi] = xT

    nats, a = _load_amax(cx, nc, x, mi * P)
    s2 = cx.scale.tile([P, KBF], F32, tag="s2x")
    rs2 = cx.scale.tile([P, KBF], F32, tag="rs2x")
    nc.vector.tensor_scalar(
        s2[:], a[:], 1e-12, 1.0 / 224.0,
        op0=mybir.AluOpType.max, op1=mybir.AluOpType.mult,
    )
    nc.vector.reciprocal(rs2[:], s2[:])
    dq = cx.dq.tile([P, K], BF16, tag="dq")
    _qdq(cx, nc, nats, dq, s2[:], rs2[:])
    return (xT, dq)


def _x_transpose(cx, nc, pend):
    xT, dq = pend
    nc.scalar.dma_start(xT[:], dq[:], transpose=True)


def _w_load_stage(cx, nc, w, wt):
    """Stage A for w row tile wt: load + chunk amax."""
    nats, a = _load_amax(cx, nc, w, wt * P)
    return {"nats": nats, "a": a}


def _w_scale_stage(cx, nc, st):
    """Stage B (emitted after a sweep, so the PE-transpose's amax input has
    had a full sweep to land): 64x64 block scales on the tiny [KBF, 2]
    tiles. The DRAM bounce is deferred (_w_bounce) to the front of the
    next iteration's ring traffic."""
    at_ps = cx.tpsum.tile([KBF, P], F32, tag="at_ps")
    nc.tensor.transpose(at_ps[:], st["a"][:], cx.ident_f32[:])
    r = cx.amax.tile([KBF, 2], F32, tag="r_blk")
    nc.vector.tensor_reduce(
        r[:], at_ps[:].rearrange("p (g t) -> p g t", t=QT),
        axis=mybir.AxisListType.X, op=mybir.AluOpType.max,
    )
    # sblk[:, nb, 0] = s2 = max(amax,eps)/224 ; sblk[:, nb, 1] = 1/s2
    sblk = cx.amax.tile([KBF, 2, 2], F32, tag="sblk")
    nc.vector.tensor_scalar(
        sblk[:, :, 0], r[:], 1e-12, 1.0 / 224.0,
        op0=mybir.AluOpType.max, op1=mybir.AluOpType.mult,
    )
    nc.vector.reciprocal(sblk[:, :, 1], sblk[:, :, 0])
    st["sblk"] = sblk


def _w_bounce(cx, nc, st):
    """Deferred part of stage B: bounce s2/1/s2 through DRAM into the
    partition-broadcast [P, KBF, 2] tile."""
    sblk = st["sblk"]
    srs = cx.scale.tile([P, KBF, 2], F32, tag="srsw")
    for nb in (0, 1):
        srow = cx.dram_small.tile([1, 2 * KBF], F32, tag="srow")
        nc.sync.dma_start(srow[:], sblk[:, nb, :])
        nc.sync.dma_start(
            srs[nb * QT : (nb + 1) * QT, :, :],
            srow[:].rearrange("o (k j) -> o k j", j=2).to_broadcast((QT, KBF, 2)),
        )
    st["srs"] = srs


def _w_qdq_stage(cx, nc, st, ns, wTp):
    """Stage C: fp8 round-trip. The transpose is emitted separately (next
    iteration, first on the sync ring) via the returned closure args."""
    srs = st["srs"]
    dq = cx.dq.tile([P, K], BF16, tag="dq")
    _qdq(cx, nc, st["nats"], dq, srs[:, :, 0], srs[:, :, 1])
    return (dq, ns, wTp)


def _w_transpose(cx, nc, pend):
    dq, ns, wTp = pend
    nc.scalar.dma_start(wTp[:, 0:KB, ts(ns, P)], dq[:], transpose=True)


def _alloc_wT(cx, nc, pn):
    wTp = cx.wT.tile([P, KB1, N_PANEL], BF16, tag="wT")
    nc.scalar.memzero(wTp[:, KB, :])
    nc.sync.dma_start(
        wTp[0:1, KB, :],
        cx.bias_bf16[pn * WR : (pn + 1) * WR, :],
    )
    return wTp


def _emit_sweep(cx, nc, out, mi, pn, wTp):
    """33 matmuls + ACT evac; returns deferred store args."""
    ps = cx.mpsum.tile([P, N_PANEL], F32, tag="mpsum")
    xT = cx.xT_units[mi]
    for kb in range(KB):
        nc.tensor.matmul(
            ps[:], xT[:, kb, :], wTp[:, kb, :], start=(kb == 0), stop=False
        )
    # bias k-slice: shared e0 lhsT adds b[n] (wTp slice KB row 0 = bias)
    nc.tensor.matmul(ps[:], cx.e0[:], wTp[:, KB, :], start=False, stop=True)
    ev = cx.evac.tile([P, N_PANEL], BF16, tag="evac")
    nc.scalar.copy(ev[:], ps[:])
    return (ev, mi, pn)


def _emit_sweep_pair(cx, nc, out, mi, pn, wTp):
    """Two m-tiles' accumulations interleaved MM-by-MM across two PSUM
    banks: one 66-MM unbroken PE stream (fewer HAM micro-idle boundaries),
    with the two stationaries ping-ponging the PE weight buffers."""
    psA = cx.mpsum.tile([P, N_PANEL], F32, tag="mpsum")
    psB = cx.mpsum.tile([P, N_PANEL], F32, tag="mpsum")
    xA = cx.xT_units[mi]
    xB = cx.xT_units[mi + 1]
    for kb in range(KB):
        nc.tensor.matmul(
            psA[:], xA[:, kb, :], wTp[:, kb, :], start=(kb == 0), stop=False
        )
        nc.tensor.matmul(
            psB[:], xB[:, kb, :], wTp[:, kb, :], start=(kb == 0), stop=False
        )
    nc.tensor.matmul(psA[:], cx.e0[:], wTp[:, KB, :], start=False, stop=True)
    nc.tensor.matmul(psB[:], cx.e0[:], wTp[:, KB, :], start=False, stop=True)
    evA = cx.evac.tile([P, N_PANEL], BF16, tag="evac")
    nc.scalar.copy(evA[:], psA[:])
    evB = cx.evac.tile([P, N_PANEL], BF16, tag="evac")
    nc.scalar.copy(evB[:], psB[:])
    return [(evA, mi, pn), (evB, mi + 1, pn)]


def _emit_store(cx, nc, out, pend):
    ev, mi, pn = pend
    nc.gpsimd.dma_start(out[ts(mi, P), ts(pn, N_PANEL)], ev[:])


def fp8_linear_core_kernel(tc, out, x, w, b):
    """Per-core: out [M_SH, N] bf16 = bf16(xq @ wq.T + b).
    x [M_SH, K] f32, w [N, K] f32, b [32, 128] f32 (= bias reshaped)."""
    nc = tc.nc
    ctx = tc.ctx

    cx = Ctx()
    cx.nat = ctx.enter_context(tc.tile_pool(name="nat", bufs=5))
    cx.q = ctx.enter_context(tc.tile_pool(name="q", bufs=2))
    cx.dq = ctx.enter_context(tc.tile_pool(name="dq", bufs=3))
    cx.amax = ctx.enter_context(tc.tile_pool(name="amax", bufs=3))
    cx.scale = ctx.enter_context(tc.tile_pool(name="scale", bufs=2))
    cx.xT = ctx.enter_context(tc.tile_pool(name="xT", bufs=MTILES))
    cx.wT = ctx.enter_context(tc.tile_pool(name="wT", bufs=2))
    cx.mpsum = ctx.enter_context(tc.tile_pool(name="mpsum", bufs=6, space="PSUM"))
    cx.tpsum = ctx.enter_context(tc.tile_pool(name="tpsum", bufs=2, space="PSUM"))
    cx.evac = ctx.enter_context(tc.tile_pool(name="evac", bufs=4))
    cx.const = ctx.enter_context(tc.tile_pool(name="const", bufs=1))
    cx.dram_small = ctx.enter_context(
        tc.tile_pool(name="scratch_s", bufs=8, space="DRAM")
    )
    cx.xT_units = [None] * MTILES

    cx.ident_f32 = cx.const.tile([P, P], F32, tag="ident")
    make_identity(nc, cx.ident_f32)

    # shared bias lhsT: e0 pattern (ones on partition 0, zero elsewhere)
    cx.e0 = cx.const.tile([P, P], BF16, tag="e0")
    nc.scalar.memzero(cx.e0[:])
    nc.scalar.add(cx.e0[0:1, :], cx.e0[0:1, :], 1.0)

    # bias as bf16 in DRAM scratch, laid out [32, 128] row-major = b[4096]
    bt = cx.const.tile([32, P], F32, tag="bt")
    nc.sync.dma_start(bt[:], b)
    btb = cx.const.tile([32, P], BF16, tag="btb")
    nc.vector.tensor_copy(btb[:], bt[:])
    bias_dram = cx.dram_small.tile([32, P], BF16, tag="bias_dram")
    nc.gpsimd.dma_start(bias_dram[:], btb[:])
    cx.bias_bf16 = bias_dram

    # ---- production + sweeps ----
    # w row tiles stream through a 3-stage pipeline woven between sweeps:
    #   iter mi:   A(r_mi)  load + chunk amax          (before the sweep)
    #              C(r_mi-1) fp8 round-trip + transpose (before the sweep)
    #              sweep mi                             (33 matmuls)
    #              B(r_mi)  PE-transpose + block scales + DRAM bounce
    # The B-stage PE-transpose queues AFTER the sweep's matmuls, so its
    # amax has a full sweep to land and never head-of-line-blocks the PE;
    # the bounce flies during the next iteration's A-stage amax.
    # Prologue: panels 0 and 1 produced up front (PE idle anyway), with the
    # one-row-tile skew hiding each bounce under the next tile's amax.
    staged = {}

    # Phase 1: w0's four row tiles pipeline 1-deep skewed (bounce of tile i
    # hides under the amax of tile i+1); then x units stream with w1's row
    # tiles woven 2:1 -- every w bounce flies under the next x unit's amax.
    # Panel-0 sweeps chase the x units.
    wT0 = _alloc_wT(cx, nc, 0)
    staged[0] = _w_load_stage(cx, nc, w, 0)
    _w_scale_stage(cx, nc, staged[0])
    for i in range(1, WR):
        staged[i] = _w_load_stage(cx, nc, w, i)
        _w_scale_stage(cx, nc, staged[i])
        _w_bounce(cx, nc, staged[i - 1])
        _w_transpose(cx, nc, _w_qdq_stage(cx, nc, staged.pop(i - 1), i - 1, wT0))
    _w_bounce(cx, nc, staged[WR - 1])
    _w_transpose(cx, nc, _w_qdq_stage(cx, nc, staged.pop(WR - 1), WR - 1, wT0))

    wT1 = _alloc_wT(cx, nc, 1)
    pend_xt = None
    pend_s = []
    wst = None
    for mi in range(MTILES):
        if pend_xt is not None:
            _x_transpose(cx, nc, pend_xt)
        for s in pend_s:
            _emit_store(cx, nc, out, s)
        pend_s = []
        # w1 row tile mi//2: qdq after its bounce flew under the previous
        # x unit's amax
        if mi % 2 == 0 and mi // 2 < WR:
            if wst is not None:
                _w_bounce(cx, nc, wst)
                _w_transpose(cx, nc, _w_qdq_stage(cx, nc, wst, (mi // 2) - 1, wT1))
            wst = _w_load_stage(cx, nc, w, WR + mi // 2)
        pend_xt = _emit_x_unit(cx, nc, x, mi)
        if mi % 2 == 1 and mi > 1:
            pend_s = _emit_sweep_pair(cx, nc, out, mi - 3, 0, wT0)
        if mi % 2 == 0 and mi // 2 < WR:
            _w_scale_stage(cx, nc, wst)
    _w_bounce(cx, nc, wst)
    _w_transpose(cx, nc, _w_qdq_stage(cx, nc, wst, WR - 1, wT1))
    _x_transpose(cx, nc, pend_xt)
    pend_s2 = _emit_sweep_pair(cx, nc, out, MTILES - 2, 0, wT0)

    # Steady state: 4 pair-sweep iterations per panel; panel p+1's row
    # tiles run the staged pipeline (A it0-3, C it1-3 + it3-end, B post-
    # sweep, bounce/transpose deferred to the next iteration's ring front).
    pend_t = pend_b = None
    wTs = {1: wT1}
    for pn in range(1, PANELS):
        nxt = pn + 1
        if nxt < PANELS:
            wTs[nxt] = _alloc_wT(cx, nc, nxt)
        for it in range(4):
            if pend_t is not None:
                _w_transpose(cx, nc, pend_t)
                pend_t = None
            if pend_b is not None:
                _w_bounce(cx, nc, pend_b)
                pend_b = None
            for s in pend_s:
                _emit_store(cx, nc, out, s)
            pend_s = []
            if pend_s2 is not None:
                for s in pend_s2:
                    _emit_store(cx, nc, out, s)
                pend_s2 = None
            if nxt < PANELS:
                staged[it] = _w_load_stage(cx, nc, w, nxt * WR + it)
                if it >= 1:
                    pend_t = _w_qdq_stage(cx, nc, staged.pop(it - 1), it - 1, wTs[nxt])
            pend_s = _emit_sweep_pair(cx, nc, out, 2 * it, pn, wTs[pn])
            if nxt < PANELS:
                _w_scale_stage(cx, nc, staged[it])
                pend_b = staged[it]
                if it == 3:
                    # finish the panel's last row tile within this panel
                    _w_bounce(cx, nc, staged[it])
                    pend_b = None
                    if pend_t is not None:
                        _w_transpose(cx, nc, pend_t)
                    pend_t = None
                    _w_transpose(
                        cx, nc, _w_qdq_stage(cx, nc, staged.pop(it), it, wTs[nxt])
                    )
    for s in pend_s:
        _emit_store(cx, nc, out, s)
    return


def build_core_bass():
    nc = bacc.Bacc(
        "TRN2", target_bir_lowering=False, debug=False, num_devices=N_CORES
    )
    x = nc.dram_tensor("x", [M_SH, K], F32, kind="ExternalInput").ap()
    w = nc.dram_tensor("w", [N, K], F32, kind="ExternalInput").ap()
    b = nc.dram_tensor("b", [32, P], F32, kind="ExternalInput").ap()
    out = nc.dram_tensor("out", [M_SH, N], BF16, kind="ExternalOutput").ap()
    with tile.TileContext(nc) as tc:
        with ExitStack() as stack:
            tc.ctx = stack
            fp8_linear_core_kernel(tc, out, x, w, b)
    nc.compile()
    return nc


_NC_CACHE = []


def _get_nc():
    if not _NC_CACHE:
        _NC_CACHE.append(build_core_bass())
    return _NC_CACHE[0]


def kernel(x, weight, bias):
    """Full-problem entry point: x [2,4096,4096] f32, weight [4096,4096] f32,
    bias [4096] f32 -> [2,4096,4096] f32."""
    from concourse.bass_utils import run_bass_kernel_spmd

    x2d = np.ascontiguousarray(x.reshape(M, K), dtype=np.float32)
    weight = np.ascontiguousarray(weight, dtype=np.float32)
    b32 = np.ascontiguousarray(bias.reshape(32, P), dtype=np.float32)

    nc = _get_nc()

    in_maps = []
    for core in range(N_CORES):
        in_maps.append(
            {
                "x": np.ascontiguousarray(x2d[core * M_SH : (core + 1) * M_SH]),
                "w": weight,
                "b": b32,
            }
        )

    res = run_bass_kernel_spmd(nc, in_maps, core_ids=list(range(N_CORES)))
    global LAST_EXEC_TIME_NS
    LAST_EXEC_TIME_NS = res.exec_time_ns

    out = np.empty((M, N), dtype=np.float32)
    for core in range(N_CORES):
        out[core * M_SH : (core + 1) * M_SH] = np.asarray(
            res.results[core]["out"]
        ).astype(np.float32)
    return out.reshape(B, S, N)
